# revision 7
# baseline (speedup 1.0000x reference)
"""Trainium2 Bass kernel for MQA cross-attention (nn_CrossAttention).

Reference computation (fp32):
    q = (x @ Wq).reshape(b, n, 16, 128).transpose(0,2,1,3) * 128**-0.5
    sim = q @ k^T   (k/v shared across heads, MQA)
    out = softmax(sim) @ v
    y = out.merge_heads @ Wo
Tolerance is rel-err < 2e-2 vs the fp32 reference; the fp16 datapath below
lands ~1e-3.

Sharding: pure sequence-parallel across 8 cores. Each core gets 256 rows
of x per batch (512 rows total), full Wq/Wo/k/v, and produces its 512 rows
of the output. No collectives, no host-side reduction.

All moving/stationary matmul operands are fp16 (same PE rate as f32r at
1 cycle/row, half the DMA bytes, and 2-byte dtypes unlock the DVE 2x mode
for the softmax row-sum accumulation). PSUM stays fp32; the softmax
denominator tail (fold/all-reduce/reciprocal) stays fp32.

Per-core schedule:
  Prologue: xt + Wq(h0,h1) + k/v DMA'd in fine chunks; qproj h0..h2.
  16 attention units (hp, b), each:
    - 8 jg iterations: 2 sim matmuls -> exp (ACT, ->fp16) -> 2 av matmuls,
      with 2 matmuls of the *next* needed head's q-projection interleaved
      per jg (keeps PE ahead of ACT/DVE),
    - DVE accumulates fp16 row-sum partials (2x mode), folds to fp32,
    - GpSimd does the 128-way partition reduce; DVE reciprocal + normalize.
    - q-projection PSUM->SBUF copies ride on GpSimd (ACT stays exp-only).
    - Wo tiles prefetch into SBUF during units 0..7 (DMA is idle then).
  Phase C: pure-PE output projection from SBUF-resident Wo, fp32 out.
"""

import sys
import numpy as np

for _p in ("/opt/trn_rl_repo", "/root/.axon_site/_ro/trn_rl_repo"):
    if _p not in sys.path:
        sys.path.append(_p)

import concourse.bass as bass  # noqa: E402
import concourse.mybir as mybir  # noqa: E402
import concourse.tile as tile  # noqa: E402
from concourse import bacc, bass_isa  # noqa: E402
from concourse.bass_utils import run_bass_kernel_spmd  # noqa: E402

F32 = mybir.dt.float32
F16 = mybir.dt.float16

B = 2
N = 2048          # query length (global)
J = 2048          # kv length
E = 2048          # model dim
HEADS = 16
DH = 128          # head dim
NCORES = 8
NC_ROWS = N // NCORES        # 256 query rows per core per batch
R = B * NC_ROWS              # 512 rows per core, col = b*NC_ROWS + i
ET = E // 128                # 16 e-tiles
FT = HEADS                   # 16 f-tiles (one per head, DH == 128)
JT = J // 128                # 16 j-tiles
SCALE = float(DH) ** -0.5

_CACHE = {}


def _build(reps: int = 1):
    nc = bacc.Bacc(name=f"mqa_xattn_r{reps}")
    xt_d = nc.declare_dram_parameter("xt", [E, R], F16, isOutput=False)
    kt_d = nc.declare_dram_parameter("kt", [B, DH, J], F16, isOutput=False)
    v_d = nc.declare_dram_parameter("v", [B, J, DH], F16, isOutput=False)
    wq_d = nc.declare_dram_parameter("wq", [E, E], F16, isOutput=False)
    wo_d = nc.declare_dram_parameter("wo", [E, E], F16, isOutput=False)
    o_d = nc.declare_dram_parameter("o", [R, E], F32, isOutput=True)

    with tile.TileContext(nc) as tc:
        for _ in range(reps):
            _emit_once(nc, tc, xt_d, kt_d, v_d, wq_d, wo_d, o_d)

    nc.compile()
    return nc


def _emit_once(nc, tc, xt_d, kt_d, v_d, wq_d, wo_d, o_d):
    with tc.tile_pool(name="persist", bufs=1) as pp:
        kt_sb = pp.tile([128, B, J], F16)
        v_sb = pp.tile([128, B, JT, DH], F16)
        qt_all = pp.tile([128, FT, R], F16)
        # free layout: [b][h][i] with i contiguous per head
        outn_all = pp.tile([128, B, FT * NC_ROWS], F16)
        # Wo resident in SBUF: [d-partition][ft][e]; loaded during phase B
        wo_sb = pp.tile([128, FT, E], F16)

        with tc.tile_pool(name="xt_pool", bufs=1) as xtp, \
             tc.tile_pool(name="wq_pool", bufs=4) as wqp, \
             tc.tile_pool(name="es_pool", bufs=4) as esp, \
             tc.tile_pool(name="rb_pool", bufs=2) as rbp, \
             tc.tile_pool(name="qp_ps", bufs=2, space="PSUM") as qp_ps, \
             tc.tile_pool(name="sg_ps", bufs=2, space="PSUM") as sg_ps, \
             tc.tile_pool(name="acc_ps", bufs=2, space="PSUM") as acc_ps:
            xt_sb = xtp.tile([128, ET, R], F16)

            wq_r = wq_d.rearrange("(et p) f -> p et f", p=128)
            xt_r = xt_d.rearrange("(et p) r -> p et r", p=128)
            kt_r = kt_d.rearrange("b p j -> p b j")
            v_r = v_d.rearrange("b (jt p) d -> p b jt d", p=128)
            wo_r = wo_d.rearrange("(ft p) e -> p ft e", p=128)

            def load_wq(h):
                wq_sb = wqp.tile([128, ET, 128], F16, tag="wq",
                                 name=f"wq_sb{h}")
                nc.sync.dma_start(wq_sb[:], wq_r[:, :, h * 128:(h + 1) * 128])
                return wq_sb

            # DMA order: head-0/1 Wq, then xt in fine et chunks (the first
            # qproj matmuls start as soon as et-0 lands), kt for batch 0
            # mid-stream, wq2 and the first half of v0 just-in-time for the
            # first attention unit, then the batch-1 tensors.
            wq_tiles = [load_wq(0), load_wq(1)]
            for et in range(ET):
                nc.sync.dma_start(xt_sb[:, et, :], xt_r[:, et, :])
                if et == 7:
                    nc.sync.dma_start(kt_sb[:, 0, :], kt_r[:, 0, :])
            nc.sync.dma_start(v_sb[:, 0, 0:8, :], v_r[:, 0, 0:8, :])
            wq_tiles.append(load_wq(2))
            nc.sync.dma_start(v_sb[:, 0, 8:JT, :], v_r[:, 0, 8:JT, :])
            nc.sync.dma_start(kt_sb[:, 1, :], kt_r[:, 1, :])
            nc.sync.dma_start(v_sb[:, 1, :, :], v_r[:, 1, :, :])

            # prologue: heads 0/1 q-projections, et-interleaved so PE
            # starts as soon as xt tile 0 lands (DMA-paced)
            q_ps0 = qp_ps.tile([128, R], F32, tag="qp")
            q_ps1 = qp_ps.tile([128, R], F32, tag="qp")
            for et in range(ET):
                nc.tensor.matmul(q_ps0[:], wq_tiles[0][:, et, :],
                                 xt_sb[:, et, :],
                                 start=(et == 0), stop=(et == ET - 1))
                nc.tensor.matmul(q_ps1[:], wq_tiles[1][:, et, :],
                                 xt_sb[:, et, :],
                                 start=(et == 0), stop=(et == ET - 1))
            with nc.allow_low_precision(reason="f16 q"):
                nc.vector.tensor_copy(qt_all[:, 0, :], q_ps0[:])
                nc.vector.tensor_copy(qt_all[:, 1, :], q_ps1[:])

            # q-projection emission for heads 2.. is spread through the
            # attention units, 2 matmuls per jg iteration.
            q_state = {"h": None, "ps": None, "et": 0}

            def qproj_start(h):
                q_state["h"] = h
                q_state["et"] = 0
                q_state["ps"] = qp_ps.tile([128, R], F32, tag="qp",
                                           name=f"q_ps{h}")
                if len(wq_tiles) < HEADS:
                    wq_tiles.append(load_wq(len(wq_tiles)))

            def qproj_step(nmm):
                """Emit nmm accumulating matmuls of the current head's
                projection; after the 16th, copy PSUM->qt_all on GpSimd."""
                h = q_state["h"]
                if h is None:
                    return
                wq_sb = wq_tiles[h]
                q_ps = q_state["ps"]
                for _ in range(nmm):
                    et = q_state["et"]
                    if et >= ET:
                        break
                    nc.tensor.matmul(q_ps[:], wq_sb[:, et, :],
                                     xt_sb[:, et, :],
                                     start=(et == 0), stop=(et == ET - 1))
                    q_state["et"] = et + 1
                if q_state["et"] >= ET:
                    with nc.allow_low_precision(reason="f16 q"):
                        nc.vector.tensor_copy(qt_all[:, h, :], q_ps[:])
                    q_state["h"] = None

            for u in range(HEADS):
                hp, b = u // 2, u % 2
                if u + 2 < HEADS:
                    qproj_start(u + 2)
                # Both heads of the pair processed together: every attention
                # matmul has a 512-wide fp16 moving operand laid out
                # [h2, i256].  PSUM start/stop groups are bank-granular, so
                # outT and the q-projection need separate banks.
                acc = acc_ps.tile([128, 512], F32, tag="acc")
                qt_pair = qt_all[:, 2 * hp:2 * hp + 2,
                                 b * NC_ROWS:(b + 1) * NC_ROWS]
                s1024 = rbp.tile([128, 1024], F16, tag="s128")
                for jg in range(JT // 2):
                    sg = sg_ps.tile([128, 1024], F32, tag="sg")
                    for kk in range(2):
                        jt = jg * 2 + kk
                        nc.tensor.matmul(
                            sg[:, kk * 512:(kk + 1) * 512],
                            kt_sb[:, b, jt * 128:(jt + 1) * 128],
                            qt_pair,
                            start=True, stop=True)
                    qproj_step(2)
                    es = esp.tile([128, 1024], F16, tag="es")
                    nc.scalar.activation(
                        es[:], sg[:], mybir.ActivationFunctionType.Exp,
                        scale=SCALE)
                    # softmax denominators: fp16 partial row-sums on DVE
                    # (2x mode; the 128-way partition reduction is on GpSimd
                    # below)
                    with nc.allow_low_precision(reason="f16 rowsum"):
                        if jg == 0:
                            nc.vector.tensor_copy(s1024[:], es[:])
                        else:
                            nc.vector.tensor_add(s1024[:], s1024[:], es[:])
                    for kk in range(2):
                        jt = jg * 2 + kk
                        esk = es[:, kk * 512:(kk + 1) * 512]
                        nc.tensor.matmul(acc[:], v_sb[:, b, jt, :],
                                         esk, start=(jt == 0),
                                         stop=(jt == JT - 1))
                # Wo prefetch: 2 ft-tiles per unit during units 0..7
                if u < 8:
                    for ft in (2 * u, 2 * u + 1):
                        nc.sync.dma_start(wo_sb[:, ft, :], wo_r[:, ft, :])
                # softmax-denominator tail, entirely off the PE stream:
                # DVE fold (fp16->fp32) -> GpSimd partition all-reduce ->
                # DVE reciprocal -> DVE normalize (fp32 acc * rb -> fp16)
                s512 = rbp.tile([128, 512], F32, tag="s512", bufs=1)
                sB = rbp.tile([128, 512], F32, tag="sB", bufs=1)
                rb_sb = rbp.tile([128, 512], F32, tag="rbs")
                with nc.allow_low_precision(reason="fold to f32"):
                    nc.vector.tensor_add(s512[:], s1024[:, 0:512],
                                         s1024[:, 512:1024])
                    nc.gpsimd.partition_all_reduce(
                        sB[:], s512[:], channels=128,
                        reduce_op=bass_isa.ReduceOp.add)
                    nc.vector.reciprocal(rb_sb[:], sB[:])
                    nc.vector.tensor_mul(
                        outn_all[:, b, 2 * hp * NC_ROWS:
                                 (2 * hp + 2) * NC_ROWS],
                        acc[:], rb_sb[:])

        # ---- Phase C: output projection (Wo already in SBUF) ----
        with tc.tile_pool(name="ost_pool", bufs=4) as ostp, \
             tc.tile_pool(name="op_ps", bufs=4, space="PSUM") as op_ps:
            def o_tile(b, rt, ec, width):
                o_ps = op_ps.tile([128, width], F32, tag="op")
                for ft in range(FT):
                    i0 = ft * NC_ROWS + rt * 128
                    nc.tensor.matmul(
                        o_ps[:], outn_all[:, b, i0:i0 + 128],
                        wo_sb[:, ft, ec * 512:ec * 512 + width],
                        start=(ft == 0), stop=(ft == FT - 1))
                o_sb = ostp.tile([128, width], F32, tag="ost")
                nc.vector.tensor_copy(o_sb[:], o_ps[:])
                nc.sync.dma_start(
                    o_d[b * NC_ROWS + rt * 128:
                        b * NC_ROWS + (rt + 1) * 128,
                        ec * 512:ec * 512 + width],
                    o_sb[:])

            for ec in range(4):
                for b in range(B):
                    for rt in range(2):
                        last = (ec == 3 and b == 1 and rt == 1)
                        if not last:
                            o_tile(b, rt, ec, 512)
            # final tile split into two 256-wide halves so the closing
            # copy+store tail is short
            for half in range(2):
                o_ps = op_ps.tile([128, 256], F32, tag="opl")
                for ft in range(FT):
                    i0 = ft * NC_ROWS + 128
                    nc.tensor.matmul(
                        o_ps[:], outn_all[:, 1, i0:i0 + 128],
                        wo_sb[:, ft, 3 * 512 + half * 256:
                              3 * 512 + (half + 1) * 256],
                        start=(ft == 0), stop=(ft == FT - 1))
                o_sb = ostp.tile([128, 256], F32, tag="ostl")
                nc.vector.tensor_copy(o_sb[:], o_ps[:])
                nc.sync.dma_start(
                    o_d[NC_ROWS + 128:NC_ROWS + 256,
                        3 * 512 + half * 256:3 * 512 + (half + 1) * 256],
                    o_sb[:])


def _get_nc(reps: int = 1):
    if reps not in _CACHE:
        _CACHE[reps] = _build(reps)
    return _CACHE[reps]


def _make_in_maps(x, k, v, Wq, Wo):
    kt = np.ascontiguousarray(k.transpose(0, 2, 1)).astype(np.float16)
    v_c = np.ascontiguousarray(v).astype(np.float16)
    wq = np.ascontiguousarray(Wq).astype(np.float16)
    wo = np.ascontiguousarray(Wo).astype(np.float16)
    in_maps = []
    for c in range(NCORES):
        xs = x[:, c * NC_ROWS:(c + 1) * NC_ROWS, :]
        xt = np.ascontiguousarray(
            np.concatenate([xs[0].T, xs[1].T], axis=1)).astype(np.float16)
        in_maps.append({"xt": xt, "kt": kt, "v": v_c, "wq": wq, "wo": wo})
    return in_maps


def run_on_device(x, k, v, Wq, Wo, reps: int = 1):
    nc = _get_nc(reps)
    in_maps = _make_in_maps(x, k, v, Wq, Wo)
    res = run_bass_kernel_spmd(nc, in_maps, list(range(NCORES)))
    parts = [res.results[c]["o"].reshape(B, NC_ROWS, E) for c in range(NCORES)]
    return np.concatenate(parts, axis=1)


def kernel(x, k, v, Wq, Wo):
    x = np.asarray(x, dtype=np.float32)
    k = np.asarray(k, dtype=np.float32)
    v = np.asarray(v, dtype=np.float32)
    Wq = np.asarray(Wq, dtype=np.float32)
    Wo = np.asarray(Wo, dtype=np.float32)
    return run_on_device(x, k, v, Wq, Wo, reps=1)


# revision 14
# speedup vs baseline: 1.1032x; 1.1032x over previous
"""Trainium2 Bass kernel for MQA cross-attention (nn_CrossAttention).

Reference computation (fp32):
    q = (x @ Wq).reshape(b, n, 16, 128).transpose(0,2,1,3) * 128**-0.5
    sim = q @ k^T   (k/v shared across heads, MQA)
    out = softmax(sim) @ v
    y = out.merge_heads @ Wo
Tolerance is rel-err < 2e-2 vs the fp32 reference; the fp16 datapath below
lands ~1e-3.

Sharding: pure sequence-parallel across 8 cores. Each core gets 256 rows
of x per batch (512 rows total), full Wq/Wo/k/v, and produces its 512 rows
of the output. No collectives, no host-side reduction.

All moving/stationary matmul operands are fp16 (same PE rate as f32r at
1 cycle/row, half the DMA bytes, and 2-byte dtypes unlock the DVE 2x mode
for the softmax row-sum accumulation). PSUM stays fp32; the softmax
denominator tail (fold/all-reduce/reciprocal) stays fp32.

Per-core schedule:
  Prologue: xt + Wq(h0,h1) + k/v DMA'd in fine chunks; qproj h0..h2.
  16 attention units (hp, b), each:
    - 8 jg iterations: 2 sim matmuls -> exp (ACT, ->fp16) -> 2 av matmuls,
      with 2 matmuls of the *next* needed head's q-projection interleaved
      per jg (keeps PE ahead of ACT/DVE),
    - DVE accumulates fp16 row-sum partials (2x mode), folds to fp32,
    - GpSimd does the 128-way partition reduce; DVE reciprocal + normalize.
    - q-projection PSUM->SBUF copies ride on GpSimd (ACT stays exp-only).
    - Wo tiles prefetch into SBUF during units 0..7 (DMA is idle then).
  Phase C: pure-PE output projection from SBUF-resident Wo, fp32 out.
"""

import sys
import numpy as np

for _p in ("/opt/trn_rl_repo", "/root/.axon_site/_ro/trn_rl_repo"):
    if _p not in sys.path:
        sys.path.append(_p)

import concourse.bass as bass  # noqa: E402
import concourse.mybir as mybir  # noqa: E402
import concourse.tile as tile  # noqa: E402
from concourse import bacc, bass_isa  # noqa: E402
from concourse.bass_utils import run_bass_kernel_spmd  # noqa: E402

F32 = mybir.dt.float32
F16 = mybir.dt.float16

B = 2
N = 2048          # query length (global)
J = 2048          # kv length
E = 2048          # model dim
HEADS = 16
DH = 128          # head dim
NCORES = 8
NC_ROWS = N // NCORES        # 256 query rows per core per batch
R = B * NC_ROWS              # 512 rows per core, col = b*NC_ROWS + i
ET = E // 128                # 16 e-tiles
FT = HEADS                   # 16 f-tiles (one per head, DH == 128)
JT = J // 128                # 16 j-tiles
SCALE = float(DH) ** -0.5

_CACHE = {}


def _build(reps: int = 1):
    nc = bacc.Bacc(name=f"mqa_xattn_r{reps}")
    # wq/v are host-relaid so every DMA line is >=4KB contiguous per
    # partition (strided 256B lines run ~2x slower)
    xt_d = nc.declare_dram_parameter("xt", [E, R], F16, isOutput=False)
    kt_d = nc.declare_dram_parameter("kt", [B, DH, J], F16, isOutput=False)
    v_d = nc.declare_dram_parameter("v", [128, B, JT, DH], F16,
                                    isOutput=False)
    wq_d = nc.declare_dram_parameter("wq", [128, HEADS, ET, 128], F16,
                                     isOutput=False)
    wo_d = nc.declare_dram_parameter("wo", [E, E], F16, isOutput=False)
    o_d = nc.declare_dram_parameter("o", [R, E], F32, isOutput=True)

    with tile.TileContext(nc) as tc:
        for _ in range(reps):
            _emit_once(nc, tc, xt_d, kt_d, v_d, wq_d, wo_d, o_d)

    nc.compile()
    return nc


def _emit_once(nc, tc, xt_d, kt_d, v_d, wq_d, wo_d, o_d):
    with tc.tile_pool(name="persist", bufs=1) as pp:
        kt_sb = pp.tile([128, B, J], F16)
        v_sb = pp.tile([128, B, JT, DH], F16)
        qt_all = pp.tile([128, FT, R], F16)
        # free layout: [b][h][i] with i contiguous per head
        outn_all = pp.tile([128, B, FT * NC_ROWS], F16)
        # Wo resident in SBUF: [d-partition][ft][e]; loaded during phase B
        wo_sb = pp.tile([128, FT, E], F16)

        with tc.tile_pool(name="qp_ps", bufs=2, space="PSUM") as qp_ps, \
             tc.tile_pool(name="ost_pool", bufs=4) as ostp:
          with tc.tile_pool(name="xt_pool", bufs=1) as xtp, \
             tc.tile_pool(name="wq_pool", bufs=4) as wqp, \
             tc.tile_pool(name="es_pool", bufs=4) as esp, \
             tc.tile_pool(name="rb_pool", bufs=2) as rbp, \
             tc.tile_pool(name="sg_ps", bufs=2, space="PSUM") as sg_ps, \
             tc.tile_pool(name="acc_ps", bufs=2, space="PSUM") as acc_ps:
            xt_sb = xtp.tile([128, ET, R], F16)

            xt_r = xt_d.rearrange("(et p) r -> p et r", p=128)
            kt_r = kt_d.rearrange("b p j -> p b j")
            wo_r = wo_d.rearrange("(ft p) e -> p ft e", p=128)

            wq_tiles = []

            def load_wq(h, et0=0, et1=ET):
                if et0 == 0:
                    wq_sb = wqp.tile([128, ET, 128], F16, tag="wq",
                                     name=f"wq_sb{h}")
                    wq_tiles.append(wq_sb)
                wq_sb = wq_tiles[h]
                nc.sync.dma_start(wq_sb[:, et0:et1, :],
                                  wq_d[:, h, et0:et1, :])
                return wq_sb

            # DMA order: xt et-chunks interleaved with head-0/1 Wq chunks
            # (first qproj matmuls start as soon as et-0 of both lands),
            # kt/v for batch 0 just-in-time for the first attention unit,
            # then the batch-1 tensors.
            load_wq(0, 0, 4)
            load_wq(1, 0, 4)
            for c in range(4):
                nc.sync.dma_start(xt_sb[:, 4 * c:4 * (c + 1), :],
                                  xt_r[:, 4 * c:4 * (c + 1), :])
                if c == 0:
                    load_wq(0, 4, ET)
                    load_wq(1, 4, ET)
                if c == 1:
                    nc.sync.dma_start(kt_sb[:, 0, :], kt_r[:, 0, :])
            nc.sync.dma_start(v_sb[:, 0, 0:8, :], v_d[:, 0, 0:8, :])
            load_wq(2)
            nc.sync.dma_start(v_sb[:, 0, 8:JT, :], v_d[:, 0, 8:JT, :])
            nc.sync.dma_start(kt_sb[:, 1, :], kt_r[:, 1, :])
            nc.sync.dma_start(v_sb[:, 1, :, :], v_d[:, 1, :, :])

            # prologue: heads 0/1 q-projections, et-interleaved so PE
            # starts as soon as xt tile 0 lands (DMA-paced)
            q_ps0 = qp_ps.tile([128, R], F32, tag="qp")
            q_ps1 = qp_ps.tile([128, R], F32, tag="qp")
            for et in range(ET):
                nc.tensor.matmul(q_ps0[:], wq_tiles[0][:, et, :],
                                 xt_sb[:, et, :],
                                 start=(et == 0), stop=(et == ET - 1))
                nc.tensor.matmul(q_ps1[:], wq_tiles[1][:, et, :],
                                 xt_sb[:, et, :],
                                 start=(et == 0), stop=(et == ET - 1))
            with nc.allow_low_precision(reason="f16 q"):
                nc.vector.tensor_copy(qt_all[:, 0, :], q_ps0[:])
                nc.vector.tensor_copy(qt_all[:, 1, :], q_ps1[:])

            # q-projection emission for heads 2.. is spread through the
            # attention units, 2 matmuls per jg iteration.
            q_state = {"h": None, "ps": None, "et": 0}

            def qproj_start(h):
                q_state["h"] = h
                q_state["et"] = 0
                q_state["ps"] = qp_ps.tile([128, R], F32, tag="qp",
                                           name=f"q_ps{h}")
                if len(wq_tiles) < HEADS:
                    load_wq(len(wq_tiles))

            def qproj_step(nmm):
                """Emit nmm accumulating matmuls of the current head's
                projection; after the 16th, copy PSUM->qt_all on GpSimd."""
                h = q_state["h"]
                if h is None:
                    return
                wq_sb = wq_tiles[h]
                q_ps = q_state["ps"]
                for _ in range(nmm):
                    et = q_state["et"]
                    if et >= ET:
                        break
                    nc.tensor.matmul(q_ps[:], wq_sb[:, et, :],
                                     xt_sb[:, et, :],
                                     start=(et == 0), stop=(et == ET - 1))
                    q_state["et"] = et + 1
                if q_state["et"] >= ET:
                    with nc.allow_low_precision(reason="f16 q"):
                        nc.vector.tensor_copy(qt_all[:, h, :], q_ps[:])
                    q_state["h"] = None

            def o_tile(b, rt, ec, width=512, ec_off=0):
                """One output-projection tile: 16 accumulating matmuls from
                SBUF-resident outn/Wo into a qp-pool PSUM bank, then
                DVE copy + store."""
                o_ps = qp_ps.tile([128, R], F32, tag="qp",
                                  name=f"o_ps{b}{rt}{ec}{ec_off}")
                for ft in range(FT):
                    i0 = ft * NC_ROWS + rt * 128
                    nc.tensor.matmul(
                        o_ps[:, 0:width], outn_all[:, b, i0:i0 + 128],
                        wo_sb[:, ft, ec * 512 + ec_off:
                              ec * 512 + ec_off + width],
                        start=(ft == 0), stop=(ft == FT - 1))
                o_sb = ostp.tile([128, 512], F32, tag="ost")
                nc.vector.tensor_copy(o_sb[:, 0:width], o_ps[:, 0:width])
                nc.sync.dma_start(
                    o_d[b * NC_ROWS + rt * 128:
                        b * NC_ROWS + (rt + 1) * 128,
                        ec * 512 + ec_off:ec * 512 + ec_off + width],
                    o_sb[:, 0:width])

            for u in range(HEADS):
                hp, b = u // 2, u % 2
                if u + 2 < HEADS:
                    qproj_start(u + 2)
                # Both heads of the pair processed together: every attention
                # matmul has a 512-wide fp16 moving operand laid out
                # [h2, i256].  PSUM start/stop groups are bank-granular, so
                # outT and the q-projection need separate banks.
                acc = acc_ps.tile([128, 512], F32, tag="acc")
                qt_pair = qt_all[:, 2 * hp:2 * hp + 2,
                                 b * NC_ROWS:(b + 1) * NC_ROWS]
                s1024 = rbp.tile([128, 1024], F16, tag="s128")
                for jg in range(JT // 2):
                    sg = sg_ps.tile([128, 1024], F32, tag="sg")
                    for kk in range(2):
                        jt = jg * 2 + kk
                        nc.tensor.matmul(
                            sg[:, kk * 512:(kk + 1) * 512],
                            kt_sb[:, b, jt * 128:(jt + 1) * 128],
                            qt_pair,
                            start=True, stop=True)
                    qproj_step(2)
                    # units 14/15 have no q-projection left to interleave
                    # (ACT-paced drain); fill PE with the first two phase-C
                    # batch-0 tiles, whose outn completed at unit 13/14
                    if u == 15 and jg in (2, 5):
                        o_tile(0, (jg - 2) // 3, 0)
                    es = esp.tile([128, 1024], F16, tag="es")
                    nc.scalar.activation(
                        es[:], sg[:], mybir.ActivationFunctionType.Exp,
                        scale=SCALE)
                    # softmax denominators: fp16 partial row-sums on DVE
                    # (2x mode; the 128-way partition reduction is on GpSimd
                    # below)
                    with nc.allow_low_precision(reason="f16 rowsum"):
                        if jg == 0:
                            nc.vector.tensor_copy(s1024[:], es[:])
                        else:
                            nc.vector.tensor_add(s1024[:], s1024[:], es[:])
                    for kk in range(2):
                        jt = jg * 2 + kk
                        esk = es[:, kk * 512:(kk + 1) * 512]
                        nc.tensor.matmul(acc[:], v_sb[:, b, jt, :],
                                         esk, start=(jt == 0),
                                         stop=(jt == JT - 1))
                # Wo prefetch: 2 ft-tiles per unit during units 0..7
                if u < 8:
                    for ft in (2 * u, 2 * u + 1):
                        nc.sync.dma_start(wo_sb[:, ft, :], wo_r[:, ft, :])
                # softmax-denominator tail, entirely off the PE stream:
                # DVE fold (fp16->fp32) -> GpSimd partition all-reduce ->
                # DVE reciprocal -> DVE normalize (fp32 acc * rb -> fp16)
                s512 = rbp.tile([128, 512], F32, tag="s512", bufs=1)
                sB = rbp.tile([128, 512], F32, tag="sB", bufs=1)
                rb_sb = rbp.tile([128, 512], F32, tag="rbs")
                with nc.allow_low_precision(reason="fold to f32"):
                    nc.vector.tensor_add(s512[:], s1024[:, 0:512],
                                         s1024[:, 512:1024])
                    nc.gpsimd.partition_all_reduce(
                        sB[:], s512[:], channels=128,
                        reduce_op=bass_isa.ReduceOp.add)
                    nc.vector.reciprocal(rb_sb[:], sB[:])
                    nc.vector.tensor_mul(
                        outn_all[:, b, 2 * hp * NC_ROWS:
                                 (2 * hp + 2) * NC_ROWS],
                        acc[:], rb_sb[:])

          # ---- Phase C: output projection (Wo already in SBUF; the first
          # two batch-0 tiles were emitted inside unit 15) ----
          for ec in range(4):
                for b in range(B):
                    for rt in range(2):
                        if ec == 0 and b == 0:
                            continue  # emitted in unit 15
                        if ec == 3 and b == 1 and rt == 1:
                            continue  # final tile split below
                        o_tile(b, rt, ec)
          # final tile split into two 256-wide halves so the closing
          # copy+store tail is short
          o_tile(1, 1, 3, width=256, ec_off=0)
          o_tile(1, 1, 3, width=256, ec_off=256)


def _get_nc(reps: int = 1):
    if reps not in _CACHE:
        _CACHE[reps] = _build(reps)
    return _CACHE[reps]


def _make_in_maps(x, k, v, Wq, Wo):
    kt = np.ascontiguousarray(k.transpose(0, 2, 1)).astype(np.float16)
    # v as [p, b, jt, d]: per-partition DMA lines are jt*d contiguous
    v_c = np.ascontiguousarray(
        v.reshape(B, JT, 128, DH).transpose(2, 0, 1, 3)).astype(np.float16)
    # wq as [p, h, et, f]: per-head loads are et*f contiguous per partition
    wq = np.ascontiguousarray(
        Wq.reshape(ET, 128, HEADS, 128).transpose(1, 2, 0, 3)
    ).astype(np.float16)
    wo = np.ascontiguousarray(Wo).astype(np.float16)
    in_maps = []
    for c in range(NCORES):
        xs = x[:, c * NC_ROWS:(c + 1) * NC_ROWS, :]
        xt = np.ascontiguousarray(
            np.concatenate([xs[0].T, xs[1].T], axis=1)).astype(np.float16)
        in_maps.append({"xt": xt, "kt": kt, "v": v_c, "wq": wq, "wo": wo})
    return in_maps


def run_on_device(x, k, v, Wq, Wo, reps: int = 1):
    nc = _get_nc(reps)
    in_maps = _make_in_maps(x, k, v, Wq, Wo)
    res = run_bass_kernel_spmd(nc, in_maps, list(range(NCORES)))
    parts = [res.results[c]["o"].reshape(B, NC_ROWS, E) for c in range(NCORES)]
    return np.concatenate(parts, axis=1)


def kernel(x, k, v, Wq, Wo):
    x = np.asarray(x, dtype=np.float32)
    k = np.asarray(k, dtype=np.float32)
    v = np.asarray(v, dtype=np.float32)
    Wq = np.asarray(Wq, dtype=np.float32)
    Wo = np.asarray(Wo, dtype=np.float32)
    return run_on_device(x, k, v, Wq, Wo, reps=1)


# revision 19
# speedup vs baseline: 1.1042x; 1.0009x over previous
"""Trainium2 Bass kernel for MQA cross-attention (nn_CrossAttention).

Reference computation (fp32):
    q = (x @ Wq).reshape(b, n, 16, 128).transpose(0,2,1,3) * 128**-0.5
    sim = q @ k^T   (k/v shared across heads, MQA)
    out = softmax(sim) @ v
    y = out.merge_heads @ Wo
Tolerance is rel-err < 2e-2 vs the fp32 reference; the fp16 datapath below
lands ~1e-3.

Sharding: pure sequence-parallel across 8 cores. Each core gets 256 rows
of x per batch (512 rows total), full Wq/Wo/k/v, and produces its 512 rows
of the output. No collectives, no host-side reduction.

All moving/stationary matmul operands are fp16 (same PE rate as f32r at
1 cycle/row, half the DMA bytes, and 2-byte dtypes unlock the DVE 2x mode
for the softmax row-sum accumulation). PSUM stays fp32; the softmax
denominator tail (fold/all-reduce/reciprocal) stays fp32.

Per-core schedule:
  Prologue: xt + Wq(h0,h1) + k/v DMA'd in fine chunks; qproj h0..h2.
  16 attention units (hp, b), each:
    - 8 jg iterations: 2 sim matmuls -> exp (ACT, ->fp16) -> 2 av matmuls,
      with 2 matmuls of the *next* needed head's q-projection interleaved
      per jg (keeps PE ahead of ACT/DVE),
    - DVE accumulates fp16 row-sum partials (2x mode), folds to fp32,
    - GpSimd does the 128-way partition reduce; DVE reciprocal + normalize.
    - q-projection PSUM->SBUF copies ride on GpSimd (ACT stays exp-only).
    - Wo tiles prefetch into SBUF during units 0..7 (DMA is idle then).
  Phase C: pure-PE output projection from SBUF-resident Wo, fp32 out.
"""

import sys
import numpy as np

for _p in ("/opt/trn_rl_repo", "/root/.axon_site/_ro/trn_rl_repo"):
    if _p not in sys.path:
        sys.path.append(_p)

import concourse.bass as bass  # noqa: E402
import concourse.mybir as mybir  # noqa: E402
import concourse.tile as tile  # noqa: E402
from concourse import bacc, bass_isa  # noqa: E402
from concourse.bass_utils import run_bass_kernel_spmd  # noqa: E402

F32 = mybir.dt.float32
F16 = mybir.dt.float16

B = 2
N = 2048          # query length (global)
J = 2048          # kv length
E = 2048          # model dim
HEADS = 16
DH = 128          # head dim
NCORES = 8
NC_ROWS = N // NCORES        # 256 query rows per core per batch
R = B * NC_ROWS              # 512 rows per core, col = b*NC_ROWS + i
ET = E // 128                # 16 e-tiles
FT = HEADS                   # 16 f-tiles (one per head, DH == 128)
JT = J // 128                # 16 j-tiles
SCALE = float(DH) ** -0.5

_CACHE = {}


def _build(reps: int = 1):
    nc = bacc.Bacc(name=f"mqa_xattn_r{reps}")
    # wq/v are host-relaid so every DMA line is >=4KB contiguous per
    # partition (strided 256B lines run ~2x slower)
    xt_d = nc.declare_dram_parameter("xt", [E, R], F16, isOutput=False)
    kt_d = nc.declare_dram_parameter("kt", [B, DH, J], F16, isOutput=False)
    v_d = nc.declare_dram_parameter("v", [128, B, JT, DH], F16,
                                    isOutput=False)
    wq_d = nc.declare_dram_parameter("wq", [128, HEADS, ET, 128], F16,
                                     isOutput=False)
    wo_d = nc.declare_dram_parameter("wo", [E, E], F16, isOutput=False)
    o_d = nc.declare_dram_parameter("o", [R, E], F32, isOutput=True)

    with tile.TileContext(nc) as tc:
        for _ in range(reps):
            _emit_once(nc, tc, xt_d, kt_d, v_d, wq_d, wo_d, o_d)

    nc.compile()
    return nc


def _emit_once(nc, tc, xt_d, kt_d, v_d, wq_d, wo_d, o_d):
    with tc.tile_pool(name="persist", bufs=1) as pp:
        kt_sb = pp.tile([128, B, J], F16)
        v_sb = pp.tile([128, B, JT, DH], F16)
        qt_all = pp.tile([128, FT, R], F16)
        # free layout: [b][h][i] with i contiguous per head
        outn_all = pp.tile([128, B, FT * NC_ROWS], F16)
        # Wo resident in SBUF: [d-partition][ft][e]; loaded during phase B
        wo_sb = pp.tile([128, FT, E], F16)

        with tc.tile_pool(name="qp_ps", bufs=2, space="PSUM") as qp_ps, \
             tc.tile_pool(name="ost_pool", bufs=4) as ostp:
          with tc.tile_pool(name="xt_pool", bufs=1) as xtp, \
             tc.tile_pool(name="wq_pool", bufs=4) as wqp, \
             tc.tile_pool(name="es_pool", bufs=4) as esp, \
             tc.tile_pool(name="rb_pool", bufs=2) as rbp, \
             tc.tile_pool(name="sg_ps", bufs=2, space="PSUM") as sg_ps, \
             tc.tile_pool(name="acc_ps", bufs=2, space="PSUM") as acc_ps:
            xt_sb = xtp.tile([128, ET, R], F16)

            xt_r = xt_d.rearrange("(et p) r -> p et r", p=128)
            kt_r = kt_d.rearrange("b p j -> p b j")
            wo_r = wo_d.rearrange("(ft p) e -> p ft e", p=128)

            wq_tiles = []

            def load_wq(h, et0=0, et1=ET):
                if et0 == 0:
                    wq_sb = wqp.tile([128, ET, 128], F16, tag="wq",
                                     name=f"wq_sb{h}")
                    wq_tiles.append(wq_sb)
                wq_sb = wq_tiles[h]
                nc.sync.dma_start(wq_sb[:, et0:et1, :],
                                  wq_d[:, h, et0:et1, :])
                return wq_sb

            # DMA order: each transfer is as large as possible (completion
            # only becomes visible ~725ns after the data lands, so many
            # small DMAs serialize on descriptor retirement).  Heads 0+1 of
            # Wq ride in two combined chunks interleaved with the xt
            # stream; kt/v for batch 0 land just before the first
            # attention unit needs them.
            wq01 = wqp.tile([128, 2, ET, 128], F16, tag="wq01", bufs=1)
            wq_tiles.append(wq01[:, 0])
            wq_tiles.append(wq01[:, 1])
            nc.sync.dma_start(wq01[:, :, 0:4, :], wq_d[:, 0:2, 0:4, :])
            nc.sync.dma_start(xt_sb[:, 0:4, :], xt_r[:, 0:4, :])
            nc.sync.dma_start(wq01[:, :, 4:ET, :], wq_d[:, 0:2, 4:ET, :])
            nc.sync.dma_start(xt_sb[:, 4:8, :], xt_r[:, 4:8, :])
            nc.sync.dma_start(xt_sb[:, 8:12, :], xt_r[:, 8:12, :])
            nc.sync.dma_start(kt_sb[:, 0, :], kt_r[:, 0, :])
            nc.sync.dma_start(xt_sb[:, 12:ET, :], xt_r[:, 12:ET, :])
            nc.sync.dma_start(v_sb[:, 0, :, :], v_d[:, 0, :, :])
            load_wq(2)
            nc.sync.dma_start(kt_sb[:, 1, :], kt_r[:, 1, :])
            nc.sync.dma_start(v_sb[:, 1, :, :], v_d[:, 1, :, :])

            # prologue: heads 0/1 q-projections, et-interleaved so PE
            # starts as soon as xt tile 0 lands (DMA-paced)
            q_ps0 = qp_ps.tile([128, R], F32, tag="qp")
            q_ps1 = qp_ps.tile([128, R], F32, tag="qp")
            for et in range(ET):
                nc.tensor.matmul(q_ps0[:], wq_tiles[0][:, et, :],
                                 xt_sb[:, et, :],
                                 start=(et == 0), stop=(et == ET - 1))
                nc.tensor.matmul(q_ps1[:], wq_tiles[1][:, et, :],
                                 xt_sb[:, et, :],
                                 start=(et == 0), stop=(et == ET - 1))
            with nc.allow_low_precision(reason="f16 q"):
                nc.vector.tensor_copy(qt_all[:, 0, :], q_ps0[:])
                nc.vector.tensor_copy(qt_all[:, 1, :], q_ps1[:])

            # q-projection emission for heads 2.. is spread through the
            # attention units, 2 matmuls per jg iteration.
            q_state = {"h": None, "ps": None, "et": 0}

            def qproj_start(h):
                q_state["h"] = h
                q_state["et"] = 0
                q_state["ps"] = qp_ps.tile([128, R], F32, tag="qp",
                                           name=f"q_ps{h}")
                if len(wq_tiles) < HEADS:
                    load_wq(len(wq_tiles))

            def qproj_step(nmm):
                """Emit nmm accumulating matmuls of the current head's
                projection; after the 16th, copy PSUM->qt_all on GpSimd."""
                h = q_state["h"]
                if h is None:
                    return
                wq_sb = wq_tiles[h]
                q_ps = q_state["ps"]
                for _ in range(nmm):
                    et = q_state["et"]
                    if et >= ET:
                        break
                    nc.tensor.matmul(q_ps[:], wq_sb[:, et, :],
                                     xt_sb[:, et, :],
                                     start=(et == 0), stop=(et == ET - 1))
                    q_state["et"] = et + 1
                if q_state["et"] >= ET:
                    with nc.allow_low_precision(reason="f16 q"):
                        nc.vector.tensor_copy(qt_all[:, h, :], q_ps[:])
                    q_state["h"] = None

            def o_tile(b, rt, ec, width=512, ec_off=0):
                """One output-projection tile: 16 accumulating matmuls from
                SBUF-resident outn/Wo into a qp-pool PSUM bank, then
                DVE copy + store."""
                o_ps = qp_ps.tile([128, R], F32, tag="qp",
                                  name=f"o_ps{b}{rt}{ec}{ec_off}")
                for ft in range(FT):
                    i0 = ft * NC_ROWS + rt * 128
                    nc.tensor.matmul(
                        o_ps[:, 0:width], outn_all[:, b, i0:i0 + 128],
                        wo_sb[:, ft, ec * 512 + ec_off:
                              ec * 512 + ec_off + width],
                        start=(ft == 0), stop=(ft == FT - 1))
                o_sb = ostp.tile([128, 512], F32, tag="ost")
                nc.vector.tensor_copy(o_sb[:, 0:width], o_ps[:, 0:width])
                nc.sync.dma_start(
                    o_d[b * NC_ROWS + rt * 128:
                        b * NC_ROWS + (rt + 1) * 128,
                        ec * 512 + ec_off:ec * 512 + ec_off + width],
                    o_sb[:, 0:width])

            o_split = {}

            for u in range(HEADS):
                hp, b = u // 2, u % 2
                if u + 2 < HEADS:
                    qproj_start(u + 2)
                # Both heads of the pair processed together: every attention
                # matmul has a 512-wide fp16 moving operand laid out
                # [h2, i256].  PSUM start/stop groups are bank-granular, so
                # outT and the q-projection need separate banks.
                acc = acc_ps.tile([128, 512], F32, tag="acc")
                qt_pair = qt_all[:, 2 * hp:2 * hp + 2,
                                 b * NC_ROWS:(b + 1) * NC_ROWS]
                s1024 = rbp.tile([128, 1024], F16, tag="s128")
                for jg in range(JT // 2):
                    sg = sg_ps.tile([128, 1024], F32, tag="sg")
                    for kk in range(2):
                        jt = jg * 2 + kk
                        nc.tensor.matmul(
                            sg[:, kk * 512:(kk + 1) * 512],
                            kt_sb[:, b, jt * 128:(jt + 1) * 128],
                            qt_pair,
                            start=True, stop=True)
                    qproj_step(2)
                    # Units 14/15 have no q-projection left to interleave
                    # (ACT would pace them); fill PE with the first two
                    # phase-C batch-0 tiles.  At unit 14 only heads 0..13
                    # of batch-0 outn exist, so its tile accumulates
                    # ft 0..13 and unit 15 closes the group (hp7's outn
                    # lands during unit 15's first jgs).
                    if u == 14 and jg == 2:
                        o_ps = qp_ps.tile([128, R], F32, tag="qp",
                                          name="o_ps_sp")
                        for ft in range(14):
                            nc.tensor.matmul(
                                o_ps[:], outn_all[:, 0, ft * NC_ROWS:
                                                  ft * NC_ROWS + 128],
                                wo_sb[:, ft, 0:512],
                                start=(ft == 0), stop=False)
                        o_split["ps"] = o_ps
                    if u == 15 and jg == 2:
                        o_ps = o_split["ps"]
                        for ft in (14, 15):
                            nc.tensor.matmul(
                                o_ps[:], outn_all[:, 0, ft * NC_ROWS:
                                                  ft * NC_ROWS + 128],
                                wo_sb[:, ft, 0:512],
                                start=False, stop=(ft == 15))
                        o_sb = ostp.tile([128, 512], F32, tag="ost",
                                         name="o_sb_sp")
                        nc.vector.tensor_copy(o_sb[:], o_ps[:])
                        nc.sync.dma_start(o_d[0:128, 0:512], o_sb[:])
                    if u == 15 and jg == 5:
                        o_tile(0, 1, 0)
                    es = esp.tile([128, 1024], F16, tag="es")
                    nc.scalar.activation(
                        es[:], sg[:], mybir.ActivationFunctionType.Exp,
                        scale=SCALE)
                    # softmax denominators: fp16 partial row-sums on DVE
                    # (2x mode; the 128-way partition reduction is on GpSimd
                    # below)
                    with nc.allow_low_precision(reason="f16 rowsum"):
                        if jg == 0:
                            nc.vector.tensor_copy(s1024[:], es[:])
                        else:
                            nc.vector.tensor_add(s1024[:], s1024[:], es[:])
                    for kk in range(2):
                        jt = jg * 2 + kk
                        esk = es[:, kk * 512:(kk + 1) * 512]
                        nc.tensor.matmul(acc[:], v_sb[:, b, jt, :],
                                         esk, start=(jt == 0),
                                         stop=(jt == JT - 1))
                # Wo prefetch: 2 ft-tiles per unit during units 0..7
                if u < 8:
                    for ft in (2 * u, 2 * u + 1):
                        nc.sync.dma_start(wo_sb[:, ft, :], wo_r[:, ft, :])
                # softmax-denominator tail, entirely off the PE stream:
                # DVE fold (fp16->fp32) -> GpSimd partition all-reduce ->
                # DVE reciprocal -> DVE normalize (fp32 acc * rb -> fp16)
                s512 = rbp.tile([128, 512], F32, tag="s512", bufs=1)
                sB = rbp.tile([128, 512], F32, tag="sB", bufs=1)
                rb_sb = rbp.tile([128, 512], F32, tag="rbs")
                with nc.allow_low_precision(reason="fold to f32"):
                    nc.vector.tensor_add(s512[:], s1024[:, 0:512],
                                         s1024[:, 512:1024])
                    nc.gpsimd.partition_all_reduce(
                        sB[:], s512[:], channels=128,
                        reduce_op=bass_isa.ReduceOp.add)
                    nc.vector.reciprocal(rb_sb[:], sB[:])
                    nc.vector.tensor_mul(
                        outn_all[:, b, 2 * hp * NC_ROWS:
                                 (2 * hp + 2) * NC_ROWS],
                        acc[:], rb_sb[:])

          # ---- Phase C: output projection (Wo already in SBUF; the first
          # two batch-0 tiles were emitted inside unit 15) ----
          for ec in range(4):
                for b in range(B):
                    for rt in range(2):
                        if ec == 0 and b == 0:
                            continue  # emitted in unit 15
                        if ec == 3 and b == 1 and rt == 1:
                            continue  # final tile split below
                        o_tile(b, rt, ec)
          # final tile split into two 256-wide halves so the closing
          # copy+store tail is short
          o_tile(1, 1, 3, width=256, ec_off=0)
          o_tile(1, 1, 3, width=256, ec_off=256)


def _get_nc(reps: int = 1):
    if reps not in _CACHE:
        _CACHE[reps] = _build(reps)
    return _CACHE[reps]


def _make_in_maps(x, k, v, Wq, Wo):
    kt = np.ascontiguousarray(k.transpose(0, 2, 1)).astype(np.float16)
    # v as [p, b, jt, d]: per-partition DMA lines are jt*d contiguous
    v_c = np.ascontiguousarray(
        v.reshape(B, JT, 128, DH).transpose(2, 0, 1, 3)).astype(np.float16)
    # wq as [p, h, et, f]: per-head loads are et*f contiguous per partition
    wq = np.ascontiguousarray(
        Wq.reshape(ET, 128, HEADS, 128).transpose(1, 2, 0, 3)
    ).astype(np.float16)
    wo = np.ascontiguousarray(Wo).astype(np.float16)
    in_maps = []
    for c in range(NCORES):
        xs = x[:, c * NC_ROWS:(c + 1) * NC_ROWS, :]
        xt = np.ascontiguousarray(
            np.concatenate([xs[0].T, xs[1].T], axis=1)).astype(np.float16)
        in_maps.append({"xt": xt, "kt": kt, "v": v_c, "wq": wq, "wo": wo})
    return in_maps


def run_on_device(x, k, v, Wq, Wo, reps: int = 1):
    nc = _get_nc(reps)
    in_maps = _make_in_maps(x, k, v, Wq, Wo)
    res = run_bass_kernel_spmd(nc, in_maps, list(range(NCORES)))
    parts = [res.results[c]["o"].reshape(B, NC_ROWS, E) for c in range(NCORES)]
    return np.concatenate(parts, axis=1)


def kernel(x, k, v, Wq, Wo):
    x = np.asarray(x, dtype=np.float32)
    k = np.asarray(k, dtype=np.float32)
    v = np.asarray(v, dtype=np.float32)
    Wq = np.asarray(Wq, dtype=np.float32)
    Wo = np.asarray(Wo, dtype=np.float32)
    return run_on_device(x, k, v, Wq, Wo, reps=1)
